# revision 4
# baseline (speedup 1.0000x reference)
"""Trainium2 Bass kernel for nn_AstraloraLayer: y = (x @ W^T) * scale + x.

x: [16384, 1024] f32, w: [1048576] f32 (W = w.reshape(1024, 1024)),
scale: [1] f32.  Data-parallel over 8 NeuronCores: each core takes 2048
tokens; w and scale are replicated; no collectives needed.

Device layout: everything is computed transposed (y^T = W @ x^T) so the
contraction dim d lands on SBUF partitions for both matmul operands with
zero on-device transposes.  The host passes x^T shards and W^T (a layout
choice made while sharding); since d_inp == d_out, the x^T tiles loaded
for the matmul double as the residual operand.  Matmuls run as float32r
(full PE rate at moving dim 512, fp32 storage).
"""

import numpy as np

_N_TOKENS = 16384
_D = 1024
_N_CORES = 8
_TOK_PER_CORE = _N_TOKENS // _N_CORES  # 2048
_TOK_BLOCK = 512
_P = 128

_cache = {}


def _apply_tile_drain_patch():
    """This walrus build rejects any instruction carrying more than one
    sync wait ("Too many sync wait commands", CoreV3 setupSyncWait), but
    Tile's wait-assignment pass freely emits multi-wait instructions.
    Two patches:

    1. Wrap TileClockWait so that after assign_waits() every instruction
       with >1 wait keeps only its last wait, with the others moved onto
       freshly inserted same-engine NoOps placed just before it.
    2. Re-emit the TileContext exit drain the same way (it waits on every
       live semaphore at once and is created after assign_waits ran).
    """
    if _cache.get("patched"):
        return
    import bass_rust
    import concourse.mybir as mybir
    from concourse import tile
    from concourse.vector_clock import ScopedClock

    _Orig = tile.TileClockWait
    _counter = [0]

    def _split_multi_waits(ordered):
        for insts in ordered.values():
            out = []
            for inst in insts:
                si = inst.sync_info
                if si is not None and len(si.on_wait) > 1:
                    waits = list(si.on_wait)
                    for w in waits[:-1]:
                        _counter[0] += 1
                        nop = mybir.InstNoOp(
                            name=f"I-wsplit-{_counter[0]}", ins=[], outs=[]
                        )
                        nop.engine = inst.engine
                        nop.bass_nofuse = True
                        nop.sync_info = bass_rust.SyncInfo(
                            on_wait=[w], on_update=[]
                        )
                        out.append(nop)
                    si.on_wait = waits[-1:]
                out.append(inst)
            insts[:] = out

    class _SplitWaitClock:
        def __init__(self, tc, ordered, **kw):
            object.__setattr__(self, "_inner", _Orig(tc, ordered, **kw))
            object.__setattr__(self, "_ordered", ordered)

        def assign_waits(self, bb):
            r = self._inner.assign_waits(bb)
            _split_multi_waits(self._ordered)
            return r

        def __getattr__(self, n):
            return getattr(object.__getattribute__(self, "_inner"), n)

    tile.TileClockWait = _SplitWaitClock

    def _drain_and_barrier(self, tick_clock, wait_clock):
        drain_inst = self.nc.sync.drain()
        wait_clock.add_sem_waits(
            drain_inst.ins, ScopedClock({None: tick_clock.global_clock})
        )
        si = drain_inst.ins.sync_info
        if si is not None and len(si.on_wait) > 1:
            waits = list(si.on_wait)
            si.on_wait = waits[:1]
            for w in waits[1:]:
                nop = self.nc.sync.nop(nofuse=True, hint="drain_wait_spill")
                nop.ins.sync_info = bass_rust.SyncInfo(on_wait=[w], on_update=[])

        self.nc.all_engine_barrier()
        assert self.sems is not None
        popped = self.nc._tile_sem_poison_stack.pop()
        assert popped is self._sem_poison
        self.nc.clear_and_free_semaphores(list(self.sems.allocated().values()))
        self.nc.all_engine_barrier()

    tile.TileContext._drain_and_barrier = _drain_and_barrier
    _cache["patched"] = True


def _build_nc():
    import concourse.bass as bass
    import concourse.mybir as mybir
    from concourse import tile

    f32 = mybir.dt.float32
    f32r = mybir.dt.float32r
    KC = _D // _P  # 8 contraction chunks
    OC = _D // _P  # 8 output-row chunks
    NB = _TOK_PER_CORE // _TOK_BLOCK  # 4 token blocks

    nc = bass.Bass()
    # xT/wT are float32r end-to-end: the BIR verifier requires every
    # producer feeding an FP32r matmul to be f32r-typed (bit layout is
    # plain fp32 either way, np side sees float32).
    xT = nc.declare_dram_parameter("xT", [_D, _TOK_PER_CORE], f32r, isOutput=False)
    wT = nc.declare_dram_parameter("wT", [_D, _D], f32r, isOutput=False)
    scale = nc.declare_dram_parameter("scale", [1], f32, isOutput=False)
    yT = nc.declare_dram_parameter("yT", [_D, _TOK_PER_CORE], f32, isOutput=True)

    with tile.TileContext(nc) as tc:
        with (
            tc.tile_pool(name="wp", bufs=1) as wp,
            tc.tile_pool(name="xp", bufs=2) as xp,
            tc.tile_pool(name="yp", bufs=3) as yp,
            tc.tile_pool(name="sp", bufs=1) as sp,
            tc.tile_pool(name="ps", bufs=4, space="PSUM") as ps,
        ):
            s128 = sp.tile([_P, 1], f32, tag="s128")
            nc.sync.dma_start(out=s128[:, :], in_=scale[:].partition_broadcast(_P))

            wts = []
            for k in range(KC):
                t = wp.tile([_P, _D], f32r, tag=f"w{k}")
                nc.sync.dma_start(out=t[:], in_=wT[k * _P : (k + 1) * _P, :])
                wts.append(t)

            for b in range(NB):
                t0 = b * _TOK_BLOCK
                xts = []
                for k in range(KC):
                    t = xp.tile([_P, _TOK_BLOCK], f32r, tag=f"x{k}")
                    nc.sync.dma_start(
                        out=t[:], in_=xT[k * _P : (k + 1) * _P, t0 : t0 + _TOK_BLOCK]
                    )
                    xts.append(t)
                for o in range(OC):
                    pt = ps.tile([_P, _TOK_BLOCK], mybir.dt.float32, tag="ps")
                    for k in range(KC):
                        nc.tensor.matmul(
                            pt[:],
                            lhsT=wts[k][:, o * _P : (o + 1) * _P],
                            rhs=xts[k][:],
                            start=(k == 0),
                            stop=(k == KC - 1),
                        )
                    yt = yp.tile([_P, _TOK_BLOCK], f32, tag="y")
                    nc.vector.scalar_tensor_tensor(
                        out=yt[:],
                        in0=pt[:],
                        scalar=s128[:, :],
                        in1=xts[o][:].bitcast(f32),
                        op0=mybir.AluOpType.mult,
                        op1=mybir.AluOpType.add,
                    )
                    nc.sync.dma_start(
                        out=yT[o * _P : (o + 1) * _P, t0 : t0 + _TOK_BLOCK], in_=yt[:]
                    )
    return nc


def kernel(x, w, scale):
    _apply_tile_drain_patch()
    from concourse.bass_utils import run_bass_kernel_spmd

    x = np.asarray(x, dtype=np.float32)
    w = np.asarray(w, dtype=np.float32)
    scale = np.asarray(scale, dtype=np.float32).reshape(1)
    wT = np.ascontiguousarray(w.reshape(_D, _D).T)

    in_maps = []
    for i in range(_N_CORES):
        xs = x[i * _TOK_PER_CORE : (i + 1) * _TOK_PER_CORE]
        in_maps.append(
            {"xT": np.ascontiguousarray(xs.T), "wT": wT, "scale": scale}
        )

    if "nc" not in _cache:
        _cache["nc"] = _build_nc()
    res = run_bass_kernel_spmd(_cache["nc"], in_maps, core_ids=list(range(_N_CORES)))

    out = np.empty((_N_TOKENS, _D), dtype=np.float32)
    for i in range(_N_CORES):
        out[i * _TOK_PER_CORE : (i + 1) * _TOK_PER_CORE] = res.results[i]["yT"].T
    return out


# revision 7
# speedup vs baseline: 1.0039x; 1.0039x over previous
"""Trainium2 Bass kernel for nn_AstraloraLayer: y = (x @ W^T) * scale + x.

x: [16384, 1024] f32, w: [1048576] f32 (W = w.reshape(1024, 1024)),
scale: [1] f32.  Data-parallel over 8 NeuronCores: each core takes 2048
tokens; w and scale are replicated; no collectives needed.

Device layout: everything is computed transposed (y^T = W @ x^T) so the
contraction dim d lands on SBUF partitions for both matmul operands with
zero on-device transposes.  The host passes x^T shards and W^T (a layout
choice made while sharding); since d_inp == d_out, the x^T tiles loaded
for the matmul double as the residual operand.

Loop order is k-outer across 8 PSUM banks (one per 128-row output chunk)
so the first matmul only needs one w chunk + one x chunk — the PE starts
~0.75 MB into the DMA stream instead of after the full 6 MB working set.
Input DMAs issue on the Sync HWDGE queue, output DMAs on the Scalar
HWDGE queue so stores never head-of-line-block loads.
"""

import numpy as np

_N_TOKENS = 16384
_D = 1024
_N_CORES = 8
_TOK_PER_CORE = _N_TOKENS // _N_CORES  # 2048
_TOK_BLOCK = 512
_P = 128

# Compute dtype for the matmul operands: "bf16" halves input DMA traffic
# (host casts the shards) and double-pumps the PE moving operand;
# "f32r" is full fp32 storage with single-pass reduced-precision matmul.
_COMPUTE = "bf16"

_cache = {}


def _apply_tile_drain_patch():
    """This walrus build rejects any instruction carrying more than one
    sync wait ("Too many sync wait commands", CoreV3 setupSyncWait), but
    Tile's wait-assignment pass freely emits multi-wait instructions.
    Two patches:

    1. Wrap TileClockWait so that after assign_waits() every instruction
       with >1 wait keeps only its last wait, with the others moved onto
       freshly inserted same-engine NoOps placed just before it.
    2. Re-emit the TileContext exit drain the same way (it waits on every
       live semaphore at once and is created after assign_waits ran).
    """
    if _cache.get("patched"):
        return
    import bass_rust
    import concourse.mybir as mybir
    from concourse import tile
    from concourse.vector_clock import ScopedClock

    _Orig = tile.TileClockWait
    _counter = [0]

    def _split_multi_waits(ordered):
        for insts in ordered.values():
            out = []
            for inst in insts:
                si = inst.sync_info
                if si is not None and len(si.on_wait) > 1:
                    waits = list(si.on_wait)
                    for w in waits[:-1]:
                        _counter[0] += 1
                        nop = mybir.InstNoOp(
                            name=f"I-wsplit-{_counter[0]}", ins=[], outs=[]
                        )
                        nop.engine = inst.engine
                        nop.bass_nofuse = True
                        nop.sync_info = bass_rust.SyncInfo(
                            on_wait=[w], on_update=[]
                        )
                        out.append(nop)
                    si.on_wait = waits[-1:]
                out.append(inst)
            insts[:] = out

    class _SplitWaitClock:
        def __init__(self, tc, ordered, **kw):
            object.__setattr__(self, "_inner", _Orig(tc, ordered, **kw))
            object.__setattr__(self, "_ordered", ordered)

        def assign_waits(self, bb):
            r = self._inner.assign_waits(bb)
            _split_multi_waits(self._ordered)
            return r

        def __getattr__(self, n):
            return getattr(object.__getattribute__(self, "_inner"), n)

    tile.TileClockWait = _SplitWaitClock

    def _drain_and_barrier(self, tick_clock, wait_clock):
        drain_inst = self.nc.sync.drain()
        wait_clock.add_sem_waits(
            drain_inst.ins, ScopedClock({None: tick_clock.global_clock})
        )
        si = drain_inst.ins.sync_info
        if si is not None and len(si.on_wait) > 1:
            waits = list(si.on_wait)
            si.on_wait = waits[:1]
            for w in waits[1:]:
                nop = self.nc.sync.nop(nofuse=True, hint="drain_wait_spill")
                nop.ins.sync_info = bass_rust.SyncInfo(on_wait=[w], on_update=[])

        self.nc.all_engine_barrier()
        assert self.sems is not None
        popped = self.nc._tile_sem_poison_stack.pop()
        assert popped is self._sem_poison
        self.nc.clear_and_free_semaphores(list(self.sems.allocated().values()))
        self.nc.all_engine_barrier()

    tile.TileContext._drain_and_barrier = _drain_and_barrier
    _cache["patched"] = True


def _build_nc(compute=None):
    import concourse.bass as bass
    import concourse.mybir as mybir
    from concourse import tile

    compute = compute or _COMPUTE
    f32 = mybir.dt.float32
    cd = mybir.dt.bfloat16 if compute == "bf16" else mybir.dt.float32r
    KC = _D // _P  # 8 contraction chunks
    OC = _D // _P  # 8 output-row chunks
    NB = _TOK_PER_CORE // _TOK_BLOCK  # token blocks

    nc = bass.Bass()
    xT = nc.declare_dram_parameter("xT", [_D, _TOK_PER_CORE], cd, isOutput=False)
    wT = nc.declare_dram_parameter("wT", [_D, _D], cd, isOutput=False)
    scale = nc.declare_dram_parameter("scale", [1], f32, isOutput=False)
    yT = nc.declare_dram_parameter("yT", [_D, _TOK_PER_CORE], f32, isOutput=True)

    with tile.TileContext(nc) as tc:
        with (
            tc.tile_pool(name="wp", bufs=1) as wp,
            tc.tile_pool(name="xp", bufs=2) as xp,
            tc.tile_pool(name="yp", bufs=4) as yp,
            tc.tile_pool(name="sp", bufs=1) as sp,
            tc.tile_pool(name="ps", bufs=1, space="PSUM") as ps,
        ):
            s128 = sp.tile([_P, 1], f32, tag="s128")
            nc.sync.dma_start(out=s128[:, :], in_=scale[:].partition_broadcast(_P))

            wts = [None] * KC
            for b in range(NB):
                t0 = b * _TOK_BLOCK
                xts = []
                for k in range(KC):
                    if b == 0:
                        wt = wp.tile([_P, _D], cd, tag=f"w{k}")
                        nc.sync.dma_start(
                            out=wt[:], in_=wT[k * _P : (k + 1) * _P, :]
                        )
                        wts[k] = wt
                    t = xp.tile([_P, _TOK_BLOCK], cd, tag=f"x{k}")
                    nc.sync.dma_start(
                        out=t[:], in_=xT[k * _P : (k + 1) * _P, t0 : t0 + _TOK_BLOCK]
                    )
                    xts.append(t)

                pts = [
                    ps.tile([_P, _TOK_BLOCK], f32, tag=f"ps{o}", name=f"ps{o}_{b}")
                    for o in range(OC)
                ]
                for k in range(KC):
                    for o in range(OC):
                        nc.tensor.matmul(
                            pts[o][:],
                            lhsT=wts[k][:, o * _P : (o + 1) * _P],
                            rhs=xts[k][:],
                            start=(k == 0),
                            stop=(k == KC - 1),
                        )
                for o in range(OC):
                    yt = yp.tile([_P, _TOK_BLOCK], f32, tag="y")
                    nc.vector.scalar_tensor_tensor(
                        out=yt[:],
                        in0=pts[o][:],
                        scalar=s128[:, :],
                        in1=xts[o][:],
                        op0=mybir.AluOpType.mult,
                        op1=mybir.AluOpType.add,
                    )
                    nc.scalar.dma_start(
                        out=yT[o * _P : (o + 1) * _P, t0 : t0 + _TOK_BLOCK], in_=yt[:]
                    )
    return nc


def _np_compute_dtype():
    if _COMPUTE == "bf16":
        import ml_dtypes

        return ml_dtypes.bfloat16
    return np.float32


def kernel(x, w, scale):
    _apply_tile_drain_patch()
    from concourse.bass_utils import run_bass_kernel_spmd

    x = np.asarray(x, dtype=np.float32)
    w = np.asarray(w, dtype=np.float32)
    scale = np.asarray(scale, dtype=np.float32).reshape(1)
    cdt = _np_compute_dtype()
    wT = np.ascontiguousarray(w.reshape(_D, _D).T).astype(cdt)

    in_maps = []
    for i in range(_N_CORES):
        xs = x[i * _TOK_PER_CORE : (i + 1) * _TOK_PER_CORE]
        in_maps.append(
            {
                "xT": np.ascontiguousarray(xs.T).astype(cdt),
                "wT": wT,
                "scale": scale,
            }
        )

    if "nc" not in _cache:
        _cache["nc"] = _build_nc()
    res = run_bass_kernel_spmd(_cache["nc"], in_maps, core_ids=list(range(_N_CORES)))

    out = np.empty((_N_TOKENS, _D), dtype=np.float32)
    for i in range(_N_CORES):
        out[i * _TOK_PER_CORE : (i + 1) * _TOK_PER_CORE] = res.results[i]["yT"].T
    return out
